# revision 4
# baseline (speedup 1.0000x reference)
import numpy as np
import jax
import jax.numpy as jnp

EPS = 1e-5

# ---------- real-valued DFT matrices (no complex dtype / FFT HLO on device) ----------
def _mats(n):
    u = np.arange(n, dtype=np.float64)
    th = 2.0 * np.pi * np.outer(u, u) / n              # [x, u]
    nh = n // 2 + 1
    v = np.arange(nh, dtype=np.float64)
    thw = 2.0 * np.pi * np.outer(u, v) / n             # [y, v]
    Fr = (np.cos(th) / n).astype(np.float32)           # fwd row-DFT  e^{-i}/n
    Fi = (-np.sin(th) / n).astype(np.float32)
    Fwr = (np.cos(thw) / n).astype(np.float32)         # fwd col-DFT  [y, v]
    Fwi = (-np.sin(thw) / n).astype(np.float32)
    Br = np.cos(th).astype(np.float32)                 # inv row-DFT  e^{+i}, no scale
    Bi = np.sin(th).astype(np.float32)
    w = np.full(nh, 2.0); w[0] = 1.0; w[-1] = 1.0
    Gc = (w[:, None] * np.cos(thw.T)).astype(np.float32)   # C2R [v, y]
    Gs = (w[:, None] * np.sin(thw.T)).astype(np.float32)
    return Fr, Fi, Fwr, Fwi, Br, Bi, Gc, Gs

_M128 = _mats(128)
_M64 = _mats(64)


def _rfft2(x, m):
    # x: (T, n, n) real -> (Xr, Xi): (T, n, n//2+1)
    Fr, Fi, Fwr, Fwi = m[0], m[1], m[2], m[3]
    W1r = jnp.matmul(Fr, x)          # (n,n)@(T,n,n) -> (T,n,n)
    W1i = jnp.matmul(Fi, x)
    Xr = jnp.matmul(W1r, Fwr) - jnp.matmul(W1i, Fwi)
    Xi = jnp.matmul(W1r, Fwi) + jnp.matmul(W1i, Fwr)
    return Xr, Xi


def _irfft2(Ar, Ai, m):
    # (T, n, nh) -> (T, n, n) real; matches jnp.fft.irfft2(norm='forward')
    Br, Bi, Gc, Gs = m[4], m[5], m[6], m[7]
    Zr = jnp.matmul(Br, Ar) - jnp.matmul(Bi, Ai)
    Zi = jnp.matmul(Br, Ai) + jnp.matmul(Bi, Ar)
    return jnp.matmul(Zr, Gc) - jnp.matmul(Zi, Gs)


def _inorm3(x, g=1.0, b=0.0):
    # x: (T, n, n); per-sample norm over last two dims, scalar affine
    mu = jnp.mean(x, axis=(-2, -1), keepdims=True)
    var = jnp.var(x, axis=(-2, -1), keepdims=True)
    return (x - mu) / jnp.sqrt(var + EPS) * g + b


def _embed(tr, ti, Ho, nh):
    # tr/ti: (T, hh, m2h) top block, (T, hh, m2h) bottom block packed -> (T, Ho, nh)
    top_r, bot_r = tr
    top_i, bot_i = ti
    T, hh, m2h = top_r.shape
    padc = nh - m2h
    midr = jnp.zeros((T, Ho - 2 * hh, nh), jnp.float32)
    def row(t, b):
        tpad = jnp.pad(t, ((0, 0), (0, 0), (0, padc)))
        bpad = jnp.pad(b, ((0, 0), (0, 0), (0, padc)))
        return jnp.concatenate([tpad, midr, bpad], axis=1)
    return row(top_r, bot_r), row(top_i, bot_i)


def _cmul(ar, ai, br, bi):
    return ar * br - ai * bi, ar * bi + ai * br


def _coda_1b(x, k_spec, k_sw, k_sb, q_spec, q_sw, q_sb, v_spec, v_sw, v_sb,
             p_spec, p_sw, p_sb, m1_spec, m1_sw, m1_sb, m2_spec, m2_sw, m2_sb,
             n1_g, n1_b, an_g, an_b, n2_g, n2_b, mo_g, mo_b):
    B, C, H, W = x.shape
    heads = k_spec.shape[1]
    T = B * C
    nh = W // 2 + 1
    tokens = x.reshape(T, H, W)
    tn = _inorm3(tokens, n1_g[0], n1_b[0])

    Xr, Xi = _rfft2(tn, _M128)                    # (T, 128, 65)

    # resample to 64x64 (Fourier truncation): rows {0..31, 96..127}, cols 0..32
    Rr = jnp.concatenate([Xr[:, :32, :33], Xr[:, 96:, :33]], axis=1)
    Ri = jnp.concatenate([Xi[:, :32, :33], Xi[:, 96:, :33]], axis=1)
    R64 = _irfft2(Rr, Ri, _M64)                   # (T, 64, 64)

    def kq_branch(spec, sw, sb):
        # spec: (1, heads, 16, 9, 2) -> per-head: (T, heads, 64, 64)
        wr, wi = spec[0, :, :, :, 0], spec[0, :, :, :, 1]     # (heads, 16, 9)
        hh = 8
        # top: X rows 0..7, wc rows 0..7 ; bottom: X rows 120..127, wc rows 8..15
        xtr, xti = Xr[:, None, :hh, :9], Xi[:, None, :hh, :9]          # (T,1,8,9)
        xbr, xbi = Xr[:, None, -hh:, :9], Xi[:, None, -hh:, :9]
        tr_, ti_ = _cmul(xtr, xti, wr[None, :, :hh], wi[None, :, :hh])  # (T,heads,8,9)
        br_, bi_ = _cmul(xbr, xbi, wr[None, :, hh:], wi[None, :, hh:])
        # embed into 64x33 grid: rows 0..7 top, rows 56..63 bottom
        z = jnp.zeros((T, heads, 48, 33), jnp.float32)
        pc = lambda a: jnp.pad(a, ((0, 0), (0, 0), (0, 0), (0, 24)))
        Ar = jnp.concatenate([pc(tr_), z, pc(br_)], axis=2).reshape(T * heads, 64, 33)
        Ai = jnp.concatenate([pc(ti_), z, pc(bi_)], axis=2).reshape(T * heads, 64, 33)
        spat = _irfft2(Ar, Ai, _M64).reshape(T, heads, 64, 64)
        return spat + sw[0][None, :, None, None] * R64[:, None] + sb[None, :, None, None]

    k = kq_branch(k_spec, k_sw, k_sb)
    q = kq_branch(q_spec, q_sw, q_sb)

    # V at full res
    wr, wi = v_spec[0, :, :, :, 0], v_spec[0, :, :, :, 1]
    hh = 8
    tr_, ti_ = _cmul(Xr[:, None, :hh, :9], Xi[:, None, :hh, :9], wr[None, :, :hh], wi[None, :, :hh])
    br_, bi_ = _cmul(Xr[:, None, -hh:, :9], Xi[:, None, -hh:, :9], wr[None, :, hh:], wi[None, :, hh:])
    zv = jnp.zeros((T, heads, 112, nh), jnp.float32)
    pcv = lambda a: jnp.pad(a, ((0, 0), (0, 0), (0, 0), (0, nh - 9)))
    VAr = jnp.concatenate([pcv(tr_), zv, pcv(br_)], axis=2).reshape(T * heads, H, nh)
    VAi = jnp.concatenate([pcv(ti_), zv, pcv(bi_)], axis=2).reshape(T * heads, H, nh)
    v = _irfft2(VAr, VAi, _M128).reshape(T, heads, H, W) \
        + v_sw[0][None, :, None, None] * tn[:, None] + v_sb[None, :, None, None]

    # attention over channels C per (batch, head)
    k2 = k.reshape(B, C, heads, 64 * 64).transpose(0, 2, 1, 3).reshape(B * heads, C, 64 * 64)
    q2 = q.reshape(B, C, heads, 64 * 64).transpose(0, 2, 1, 3).reshape(B * heads, C, 64 * 64)
    v2 = v.reshape(B, C, heads, H * W).transpose(0, 2, 1, 3).reshape(B * heads, C, H * W)
    scores = jnp.matmul(q2, k2.transpose(0, 2, 1)) / jnp.float32(np.sqrt(64.0 * 64.0))
    attn = jax.nn.softmax(scores, axis=-1)
    out = jnp.matmul(attn, v2)                                  # (B*heads, C, H*W)
    out = out.reshape(B, heads, C, H, W).transpose(0, 2, 1, 3, 4).reshape(T, heads, H, W)

    # proj FNO: spectral heads->1 + 1x1 skip
    Pr, Pi = _rfft2(out.reshape(T * heads, H, W), _M128)
    Pr = Pr.reshape(T, heads, H, nh); Pi = Pi.reshape(T, heads, H, nh)
    pwr, pwi = p_spec[:, 0, :, :, 0], p_spec[:, 0, :, :, 1]     # (heads, 32, 17)
    hh = 16
    tr_, ti_ = _cmul(Pr[:, :, :hh, :17], Pi[:, :, :hh, :17], pwr[None, :, :hh], pwi[None, :, :hh])
    br_, bi_ = _cmul(Pr[:, :, -hh:, :17], Pi[:, :, -hh:, :17], pwr[None, :, hh:], pwi[None, :, hh:])
    tr_ = tr_.sum(axis=1); ti_ = ti_.sum(axis=1)                # (T, 16, 17)
    br_ = br_.sum(axis=1); bi_ = bi_.sum(axis=1)
    PAr, PAi = _embed((tr_, br_), (ti_, bi_), H, nh)
    proj = _irfft2(PAr, PAi, _M128) \
        + jnp.tensordot(out, p_sw[:, 0], axes=([1], [0])) + p_sb[0]

    a = _inorm3(proj + tokens, an_g[0], an_b[0])
    an = _inorm3(a, n2_g[0], n2_b[0])

    def mixer_spec(z, spec):
        zr, zi = _rfft2(z, _M128)
        mwr, mwi = spec[0, 0, :, :, 0], spec[0, 0, :, :, 1]     # (32, 17)
        hh = 16
        tr_, ti_ = _cmul(zr[:, :hh, :17], zi[:, :hh, :17], mwr[None, :hh], mwi[None, :hh])
        br_, bi_ = _cmul(zr[:, -hh:, :17], zi[:, -hh:, :17], mwr[None, hh:], mwi[None, hh:])
        Ar, Ai = _embed((tr_, br_), (ti_, bi_), H, nh)
        return _irfft2(Ar, Ai, _M128)

    h = jax.nn.gelu(_inorm3(mixer_spec(an, m1_spec)) + an * m1_sw[0, 0] + m1_sb[0],
                    approximate=False)
    h = _inorm3(mixer_spec(h, m2_spec)) + h * m2_sw[0, 0] + m2_sb[0]
    o = _inorm3(h, mo_g[0], mo_b[0]) + a
    return o.reshape(B, C, H, W)


_JITTED = {}

def _get_jitted(dev):
    if dev not in _JITTED:
        _JITTED[dev] = jax.jit(_coda_1b, device=dev)
    return _JITTED[dev]


def kernel(**inputs):
    import os
    x = np.asarray(inputs['x'])
    B = x.shape[0]
    if os.environ.get('KERNEL_FORCE_CPU'):
        devs = jax.local_devices(backend='cpu')
    else:
        devs = [d for d in jax.devices() if d.platform != 'cpu'][:8]
        if not devs:
            devs = jax.devices()[:8]
    wkeys = [k for k in inputs if k != 'x']
    futs = []
    for i in range(B):
        dev = devs[i % len(devs)]
        f = _get_jitted(dev)
        args = {k: jax.device_put(np.asarray(inputs[k]), dev) for k in wkeys}
        xi = jax.device_put(np.ascontiguousarray(x[i:i + 1]), dev)
        futs.append(f(xi, **args))
    res = np.concatenate([np.asarray(o) for o in futs], axis=0)
    return res.astype(np.float32)
